# revision 1
# baseline (speedup 1.0000x reference)
"""Trainium2 Bass kernel for CompressionSDF (4,128,128,128) -> (4,128,128,128).

Structure of the computation:
  stage: 1x1-conv stack over (B,C=128,H,W): 128->64->32->16 (lrelu, lrelu, none)
  then per-voxel MLP over a z-broadcast 17-channel field: 17->32->32->16->1
  (lrelu x3, sigmoid), where channel 16 is a z linspace coordinate.

Sharding: H axis split across 8 cores (16 rows each). Per core 8192 pixels,
1,048,576 voxels.

Per-core kernel layout: voxels are packed 4-per-column: partition dim holds
4 z-groups x 32 channels; columns are (z_lo, pixel). Layer 1 is computed by a
"selector" matmul whose stationary operand holds the per-pixel conv features
(so the z-broadcast never materializes in HBM); layers 2/3/4 use block-diagonal
weight matrices. Leaky-relu evacuations ride the PSUM->SBUF copies
(DVE scalar_tensor_tensor for L1, ScalarE Prelu for L2/L3, Sigmoid for L4).
"""

import sys

sys.path.insert(0, "/opt/trn_rl_repo")

import numpy as np
from contextlib import ExitStack

import concourse.bass as bass
import concourse.tile as tile
from concourse import bacc, mybir
from concourse.bass_utils import run_bass_kernel_spmd

F32 = mybir.dt.float32
BF16 = mybir.dt.bfloat16
AF = mybir.ActivationFunctionType
ALU = mybir.AluOpType

N_CORES = 8
B, C, H, W, D = 4, 128, 128, 128, 128
HL = H // N_CORES            # h rows per core = 16
PIX = B * HL * W             # pixels per core = 8192
PB = 64                      # pixels per block
NBLK = PIX // PB             # 128 blocks
NCH = 4                      # chunks per block, 8 z_lo each -> 32 z_lo
ALPHA = 0.01                 # LeakyReLU slope

_CACHE = {}


def _build_program(trace=False):
    nc = bacc.Bacc(
        "TRN2",
        target_bir_lowering=False,
        debug=False,
        enable_asserts=False,
        num_devices=N_CORES,
    )

    def din(name, shape, dt=F32):
        return nc.dram_tensor(name, list(shape), dt, kind="ExternalInput")

    x_d = din("x_sb", (C, PIX), BF16)
    w1_d = din("w1T", (128, 64), BF16)
    b1_d = din("b1c", (64, 1))
    w2_d = din("w2T", (64, 32), BF16)
    b2_d = din("b2c", (32, 1))
    w3_d = din("w3T", (32, 16), BF16)
    b3_d = din("b3c", (16, 1))
    gw_d = din("gw", (17, 128), BF16)
    selrow_d = din("selrow", (2, 128), BF16)
    selrhs_d = din("selrhs", (66, NCH * 512), BF16)
    l2_d = din("l2T", (128, 128), BF16)
    b2t4_d = din("b2t4", (128, 1))
    l3_d = din("l3T", (128, 128), BF16)
    b3t8_d = din("b3t8", (128, 1))
    l4_d = din("l4T", (128, 8), BF16)
    mb4_d = din("mb4t8", (8, 1))
    out_d = nc.dram_tensor("out_sd", [D, PIX], F32, kind="ExternalOutput")

    with tile.TileContext(nc) as tc, ExitStack() as octx:
        cpool = octx.enter_context(tc.tile_pool(name="consts", bufs=1))

        def load(name, dram, shape, dt=F32):
            t = cpool.tile(list(shape), dt, name=name)
            nc.sync.dma_start(out=t[:], in_=dram[:])
            return t

        xt = load("xt", x_d, (C, PIX), BF16)
        w1s = load("w1s", w1_d, (128, 64), BF16)
        b1s = load("b1s", b1_d, (64, 1))
        w2s = load("w2s", w2_d, (64, 32), BF16)
        b2s = load("b2s", b2_d, (32, 1))
        w3s = load("w3s", w3_d, (32, 16), BF16)
        b3s = load("b3s", b3_d, (16, 1))
        gws = load("gws", gw_d, (17, 128), BF16)
        selrows = load("selrows", selrow_d, (2, 128), BF16)
        selrhss = load("selrhss", selrhs_d, (66, NCH * 512), BF16)
        l2s = load("l2s", l2_d, (128, 128), BF16)
        b2t4s = load("b2t4s", b2t4_d, (128, 1))
        l3s = load("l3s", l3_d, (128, 128), BF16)
        b3t8s = load("b3t8s", b3t8_d, (128, 1))
        l4s = load("l4s", l4_d, (128, 8), BF16)
        mb4s = load("mb4s", mb4_d, (8, 1))

        f1 = cpool.tile([64, PIX], BF16, name="f1")
        f2 = cpool.tile([32, PIX], BF16, name="f2")
        f3 = cpool.tile([17, PIX], BF16, name="f3")
        nc.vector.memset(f3[:], 1.0)  # row 16 stays 1.0; rows 0..15 overwritten

        # ---- stage: pointwise conv stack over pixels ----
        with tc.tile_pool(name="psA", bufs=2, space="PSUM") as psA, \
             tc.tile_pool(name="psB", bufs=2, space="PSUM") as psB, \
             tc.tile_pool(name="psC", bufs=2, space="PSUM") as psC:
            for t in range(PIX // 512):
                s = bass.ts(t, 512)
                pa = psA.tile([64, 512], F32, name="pa")
                nc.tensor.matmul(pa[:], w1s[:], xt[:, s], start=True, stop=True)
                nc.scalar.activation(f1[:, s], pa[:], AF.Prelu, bias=b1s[:], alpha=ALPHA)
                pb = psB.tile([32, 512], F32, name="pb")
                nc.tensor.matmul(pb[:], w2s[:], f1[:, s], start=True, stop=True)
                nc.scalar.activation(f2[:, s], pb[:], AF.Prelu, bias=b2s[:], alpha=ALPHA)
                pc = psC.tile([16, 512], F32, name="pc")
                nc.tensor.matmul(pc[:], w3s[:], f2[:, s], start=True, stop=True)
                nc.scalar.activation(f3[0:16, s], pc[:], AF.Identity, bias=b3s[:])

        # ---- per-voxel MLP ----
        # z row index = 32*zg + t, t = 8*chunk + j
        osd = out_d[:].rearrange("(zg t) n -> zg t n", zg=4)

        with tc.tile_pool(name="ps1", bufs=2, space="PSUM") as ps1, \
             tc.tile_pool(name="ps2", bufs=2, space="PSUM") as ps2, \
             tc.tile_pool(name="ps3", bufs=2, space="PSUM") as ps3, \
             tc.tile_pool(name="ps4", bufs=2, space="PSUM") as ps4, \
             tc.tile_pool(name="hpool", bufs=5) as hpool, \
             tc.tile_pool(name="lhsp", bufs=3) as lhsp, \
             tc.tile_pool(name="sigp", bufs=3) as sigp:
            for blk in range(NBLK):
                bs = bass.ts(blk, PB)
                # build the selector stationary operand for this pixel block:
                # rows 0..63 = per-pixel conv features g (replicated x4 over
                # z-groups, via a small matmul), rows 64..65 = z-coordinate rows
                pg = ps1.tile([PB, 128], F32, name="pg", tag="pre1")
                nc.tensor.matmul(pg[:], f3[:, bs], gws[:], start=True, stop=True)
                lhsTb = lhsp.tile([66, 128], BF16, name="lhsTb")
                nc.vector.tensor_copy(lhsTb[0:PB, :], pg[:])
                nc.sync.dma_start(out=lhsTb[PB:PB + 2, :], in_=selrows[:])

                p3 = None
                psig = None
                for c in range(NCH):
                    u, q = c // 2, c % 2
                    # L1: selector matmul -> pre1; DVE lrelu evac
                    p1 = ps1.tile([128, 512], F32, name="p1", tag="pre1")
                    nc.tensor.matmul(p1[:], lhsTb[:], selrhss[:, bass.ts(c, 512)],
                                     start=True, stop=True)
                    h1 = hpool.tile([128, 512], BF16, name="h1")
                    nc.scalar.activation(h1[:], p1[:], AF.Prelu, alpha=ALPHA)
                    # L2: block-diag matmul; DVE 2-pass lrelu evac:
                    # h2 = p2 - 0.99*min(p2+b2, 0) = lrelu(p2+b2) - b2,
                    # with the missing b2 folded into L3's bias host-side.
                    p2 = ps2.tile([128, 512], F32, name="p2")
                    nc.tensor.matmul(p2[:], l2s[:], h1[:], start=True, stop=True)
                    m2 = hpool.tile([128, 512], F32, name="m2")
                    nc.vector.tensor_scalar(m2[:], p2[:], b2t4s[:], 0.0,
                                            op0=ALU.add, op1=ALU.min)
                    h2 = hpool.tile([128, 512], BF16, name="h2")
                    nc.vector.scalar_tensor_tensor(h2[:], m2[:], -0.99, p2[:],
                                                   op0=ALU.mult, op1=ALU.add)
                    # L3: two chunks share one PSUM tile via column groups
                    if q == 0:
                        p3 = ps3.tile([128, 512], F32, name="p3")
                    nc.tensor.matmul(p3[q * 64:(q + 1) * 64, :],
                                     l3s[:, q * 64:(q + 1) * 64], h2[:],
                                     start=True, stop=True,
                                     tile_position=(0, q * 64))
                    if q == 1:
                        h3 = hpool.tile([128, 512], BF16, name="h3")
                        nc.scalar.activation(h3[:], p3[:], AF.Prelu,
                                             bias=b3t8s[:], alpha=ALPHA)
                        # L4: 8 output rows = (chunk-in-pair, z-group)
                        psig = ps4.tile([8, 512], F32, name="psig")
                        nc.tensor.matmul(psig[:], l4s[:], h3[:],
                                         start=True, stop=True)
                        sig = sigp.tile([8, 512], F32, name="sig")
                        nc.scalar.activation(sig[:], psig[:], AF.Sigmoid,
                                             bias=mb4s[:])
                        for qq in range(2):
                            cc = 2 * u + qq
                            # z = 32*zg + 8*cc + j ; sig row 4*qq+zg
                            src = sig[4 * qq:4 * qq + 4, :]
                            src = src.rearrange("p (j w) -> p j w", j=8)
                            dst = osd[:, 8 * cc:8 * (cc + 1), bs]
                            nc.sync.dma_start(out=dst, in_=src)

    nc.compile()
    return nc


def _host_inputs(x, sw1, sb1, sw2, sb2, sw3, sb3,
                 mw1, mb1, mw2, mb2, mw3, mb3, mw4, mb4):
    import ml_dtypes
    f = np.float32
    bf = ml_dtypes.bfloat16
    zt = np.linspace(-1.0, 1.0, D, dtype=np.float64)
    c1 = mw1[:, 16].astype(np.float64)
    W1f = mw1[:, :16]

    gw = np.zeros((17, 128), f)
    gw[:16, :] = np.tile(W1f.T, (1, 4))
    gw[16, :] = np.tile(mb1, 4)

    A = zt[::32]                      # z-group base coordinate, shape (4,)
    Bv = zt[:32] - zt[0]              # z_lo offset, shape (32,)
    selrow = np.zeros((2, 128), f)
    selrow[0] = np.repeat(A, 32) * np.tile(c1, 4)
    selrow[1] = np.tile(c1, 4)

    selrhs = np.zeros((66, NCH * 512), f)
    eye_tiled = np.tile(np.eye(PB, dtype=f), (1, 8))   # [64, 512], col = j*64+p
    for c in range(NCH):
        s = slice(c * 512, (c + 1) * 512)
        selrhs[:PB, s] = eye_tiled
        selrhs[PB, s] = 1.0
        selrhs[PB + 1, s] = np.repeat(Bv[8 * c:8 * c + 8], PB)

    ins = {
        "w1T": np.ascontiguousarray(sw1.T).astype(bf),
        "b1c": sb1[:, None].astype(f),
        "w2T": np.ascontiguousarray(sw2.T).astype(bf),
        "b2c": sb2[:, None].astype(f),
        "w3T": np.ascontiguousarray(sw3.T).astype(bf),
        "b3c": sb3[:, None].astype(f),
        "gw": gw.astype(bf),
        "selrow": selrow.astype(bf),
        "selrhs": selrhs.astype(bf),
        "l2T": np.kron(np.eye(4, dtype=f), mw2.T).astype(bf),
        "b2t4": np.tile(mb2, 4)[:, None].astype(f),
        "l3T": np.concatenate([np.kron(np.eye(4, dtype=f), mw3.T)] * 2,
                              axis=1).astype(bf),
        "b3t8": np.tile(mb3 + mw3 @ mb2, 8)[:, None].astype(f),
        "l4T": np.kron(np.eye(8, dtype=f), mw4.T).astype(bf),
        "mb4t8": np.full((8, 1), mb4[0], f),
    }
    in_maps = []
    for k in range(N_CORES):
        xs = x[:, :, k * HL:(k + 1) * HL, :]
        xcore = np.ascontiguousarray(
            xs.transpose(1, 0, 2, 3).reshape(C, PIX)).astype(bf)
        in_maps.append({**ins, "x_sb": xcore})
    return in_maps


def run(trace=False, **inputs):
    if "nc" not in _CACHE:
        _CACHE["nc"] = _build_program()
    nc = _CACHE["nc"]
    in_maps = _host_inputs(**inputs)
    res = run_bass_kernel_spmd(nc, in_maps, list(range(N_CORES)), trace=trace)
    out = np.empty((B, D, H, W), np.float32)
    for k in range(N_CORES):
        o = res.results[k]["out_sd"].reshape(D, B, HL, W).transpose(1, 0, 2, 3)
        out[:, :, k * HL:(k + 1) * HL, :] = o
    return out, res


def kernel(**inputs):
    out, _ = run(trace=False, **inputs)
    return out



# revision 8
# speedup vs baseline: 2.1107x; 2.1107x over previous
"""Trainium2 Bass kernel for CompressionSDF (4,128,128,128) -> (4,128,128,128).

Structure of the computation:
  stage: 1x1-conv stack over (B,C=128,H,W): 128->64->32->16 (lrelu, lrelu, none)
  then per-voxel MLP over a z-broadcast 17-channel field: 17->32->32->16->1
  (lrelu x3, sigmoid), where channel 16 is a z linspace coordinate.

Sharding: H axis split across 8 cores (16 rows each). Per core 8192 pixels,
1,048,576 voxels.

Key observation: the voxel-MLP's first layer is h1 = lrelu(W1f@g + c1*z + b1)
where the z-dependence is a pure additive bias.  With a chunk layout of
[4 z-groups x 32 ch, 512 pixels] (z = 32*zi + c for chunk c), the z term is a
per-partition bias, so h1 never needs a matmul:
  U = (W1f@W3) @ f2  (+ fold of stage-3 bias) is computed once per pixel
  (replicated x4 over partition z-groups by tiling the stationary), and per
  chunk h1 = lrelu(U + b1z[:, c]) is produced by the DVE in two bf16 passes
  (tensor_scalar add; scalar_tensor_tensor max(0.01*q, q)).
L2/L3 use block-diagonal weights (4x32x32 / 4x32x16), L4 packs 8 rows per
chunk-pair and parks 4 pairs at 32-row offsets of one PSUM tile so a single
Sigmoid evacuates 8 chunks.  Output rows are 2KB-contiguous per z.
"""

import sys

sys.path.insert(0, "/opt/trn_rl_repo")

import numpy as np
from contextlib import ExitStack

import concourse.bass as bass
import concourse.tile as tile
from concourse import bacc, mybir
from concourse.bass_utils import run_bass_kernel_spmd

F32 = mybir.dt.float32
BF16 = mybir.dt.bfloat16
AF = mybir.ActivationFunctionType
ALU = mybir.AluOpType

N_CORES = 8
B, C, H, W, D = 4, 128, 128, 128, 128
HL = H // N_CORES            # h rows per core = 16
PIX = B * HL * W             # pixels per core = 8192
PB = 512                     # pixels per block
NPB = PIX // PB              # 16 pixel blocks
ALPHA = 0.01                 # LeakyReLU slope

_CACHE = {}


def _build_program(trace=False):
    nc = bacc.Bacc(
        "TRN2",
        target_bir_lowering=False,
        debug=False,
        enable_asserts=False,
        num_devices=N_CORES,
    )

    def din(name, shape, dt=F32):
        return nc.dram_tensor(name, list(shape), dt, kind="ExternalInput")

    x_d = din("x_sb", (C, PIX), BF16)
    w1_d = din("w1T", (128, 64), BF16)
    b1_d = din("b1c", (64, 1))
    w2_d = din("w2T", (64, 32), BF16)
    b2_d = din("b2c", (32, 1))
    wU_d = din("wU", (32, 128), BF16)
    b1z_d = din("b1z", (128, 32))
    l2_d = din("l2T", (128, 128), BF16)
    b2r_d = din("b2r", (128, 1))
    l3_d = din("l3T", (128, 128), BF16)
    b3r_d = din("b3r", (128, 1))
    l4_d = din("l4T", (128, 8), BF16)
    mb4_d = din("mb4c", (128, 1))
    out_d = nc.dram_tensor("out_sd", [D, PIX], F32, kind="ExternalOutput")

    with tile.TileContext(nc) as tc, ExitStack() as octx:
        cpool = octx.enter_context(tc.tile_pool(name="consts", bufs=1))

        def load(name, dram, shape, dt=F32):
            t = cpool.tile(list(shape), dt, name=name)
            nc.sync.dma_start(out=t[:], in_=dram[:])
            return t

        w1s = load("w1s", w1_d, (128, 64), BF16)
        b1s = load("b1s", b1_d, (64, 1))
        w2s = load("w2s", w2_d, (64, 32), BF16)
        b2s = load("b2s", b2_d, (32, 1))
        wUs = load("wUs", wU_d, (32, 128), BF16)
        b1zs = load("b1zs", b1z_d, (128, 32))
        l2s = load("l2s", l2_d, (128, 128), BF16)
        b2rs = load("b2rs", b2r_d, (128, 1))
        l3s = load("l3s", l3_d, (128, 128), BF16)
        b3rs = load("b3rs", b3r_d, (128, 1))
        l4s = load("l4s", l4_d, (128, 8), BF16)
        mb4s = load("mb4s", mb4_d, (128, 1))

        xt = cpool.tile([C, PIX], BF16, name="xt")

        # out rows: z = 32*zi + rr, rr = 8*g + 2*k + q;
        # psig partition = 32*k + 4*q + zi.  src keeps a plain contiguous
        # 4-partition range (zi), dst dim0 = zi with uniform 32-row stride.
        outz = out_d[:].rearrange("(z r) n -> r z n", z=4, r=32)

        with tc.tile_pool(name="psS", bufs=2, space="PSUM") as psS, \
             tc.tile_pool(name="ps1", bufs=3, space="PSUM") as ps1, \
             tc.tile_pool(name="ps2", bufs=2, space="PSUM") as ps2, \
             tc.tile_pool(name="ps3", bufs=1, space="PSUM") as ps3, \
             tc.tile_pool(name="f1p", bufs=2) as f1p, \
             tc.tile_pool(name="f2p", bufs=2) as f2p, \
             tc.tile_pool(name="u4p", bufs=2) as u4p, \
             tc.tile_pool(name="qp", bufs=3) as qp, \
             tc.tile_pool(name="h1p", bufs=3) as h1p, \
             tc.tile_pool(name="h2p", bufs=3) as h2p, \
             tc.tile_pool(name="h3p", bufs=3) as h3p, \
             tc.tile_pool(name="sgp", bufs=2) as sgp:
            for t in range(NPB):
                pbs = bass.ts(t, PB)
                # ---- stage convs + U for this pixel block ----
                nc.sync.dma_start(out=xt[:, pbs], in_=x_d[:, pbs])
                pa = psS.tile([64, PB], F32, name="pa", tag="st")
                nc.tensor.matmul(pa[:], w1s[:], xt[:, pbs], start=True, stop=True)
                f1 = f1p.tile([64, PB], BF16, name="f1")
                nc.scalar.activation(f1[:], pa[:], AF.Prelu, bias=b1s[:], alpha=ALPHA)
                pb2 = psS.tile([32, PB], F32, name="pb2", tag="st")
                nc.tensor.matmul(pb2[:], w2s[:], f1[:], start=True, stop=True)
                f2 = f2p.tile([32, PB], BF16, name="f2")
                nc.scalar.activation(f2[:], pb2[:], AF.Prelu, bias=b2s[:], alpha=ALPHA)
                pU = psS.tile([128, PB], F32, name="pU", tag="st")
                nc.tensor.matmul(pU[:], wUs[:], f2[:], start=True, stop=True)
                u4 = u4p.tile([128, PB], BF16, name="u4")
                nc.vector.tensor_copy(u4[:], pU[:])

                # ---- per-voxel MLP: 32 chunks of (512 pix x 4 z) ----
                psig = None
                p3 = None
                for u in range(16):          # chunk pairs
                    g, k = u // 4, u % 4
                    for qq in range(2):
                        c = 2 * u + qq
                        # h1 = lrelu(U + b1z[:, c]) on DVE (bf16 2x passes)
                        q = qp.tile([128, PB], BF16, name="q")
                        nc.vector.tensor_scalar(q[:], u4[:], b1zs[:, c:c + 1],
                                                None, op0=ALU.add)
                        h1 = h1p.tile([128, PB], BF16, name="h1")
                        nc.vector.scalar_tensor_tensor(h1[:], q[:], ALPHA, q[:],
                                                       op0=ALU.mult, op1=ALU.max)
                        # L2 (block-diag) -> ACT Prelu evac
                        p2 = ps1.tile([128, PB], F32, name="p2")
                        nc.tensor.matmul(p2[:], l2s[:], h1[:], start=True, stop=True)
                        h2 = h2p.tile([128, PB], BF16, name="h2")
                        nc.scalar.activation(h2[:], p2[:], AF.Prelu, bias=b2rs[:],
                                             alpha=ALPHA)
                        # L3: pair members share one PSUM tile via column groups
                        if qq == 0:
                            p3 = ps2.tile([128, PB], F32, name="p3")
                        nc.tensor.matmul(p3[qq * 64:(qq + 1) * 64, :],
                                         l3s[:, qq * 64:(qq + 1) * 64], h2[:],
                                         start=True, stop=True,
                                         tile_position=(0, qq * 64))
                    h3 = h3p.tile([128, PB], BF16, name="h3")
                    nc.scalar.activation(h3[:], p3[:], AF.Prelu, bias=b3rs[:],
                                         alpha=ALPHA)
                    # L4: park pair u at rows 32k of the group's PSUM tile
                    if k == 0:
                        psig = ps3.tile([128, PB], F32, name="psig")
                    nc.tensor.matmul(psig[32 * k:32 * k + 8, :], l4s[:], h3[:],
                                     start=True, stop=True,
                                     tile_position=(0, 32 * k))
                    if k == 3:
                        sg = sgp.tile([128, PB], F32, name="sg")
                        nc.scalar.activation(sg[:], psig[:], AF.Sigmoid,
                                             bias=mb4s[:])
                        for kk in range(4):
                            for q2 in range(2):
                                rr = 8 * g + 2 * kk + q2
                                nc.sync.dma_start(
                                    out=outz[rr, :, pbs],
                                    in_=sg[32 * kk + 4 * q2:
                                           32 * kk + 4 * q2 + 4, :])

    nc.compile()
    return nc


def _host_inputs(x, sw1, sb1, sw2, sb2, sw3, sb3,
                 mw1, mb1, mw2, mb2, mw3, mb3, mw4, mb4):
    import ml_dtypes
    f = np.float32
    bf = ml_dtypes.bfloat16
    zt = np.linspace(-1.0, 1.0, D, dtype=np.float64)
    c1 = mw1[:, 16].astype(np.float64)          # z-coordinate column of mw1
    W1f = mw1[:, :16].astype(np.float64)

    # U = (W1f @ sw3) @ f2 ; stationary [32, 128] tiled x4 over z-groups
    WU = (W1f @ sw3.astype(np.float64))          # (32, 16+)? sw3 is (16,32)
    wU = np.tile(WU.T, (1, 4)).astype(f)         # (32, 128)

    # b1z[(zi,ch), c] = (W1f@sb3 + mb1)[ch] + c1[ch] * z(32*zi + c)
    ub = (W1f @ sb3.astype(np.float64) + mb1.astype(np.float64))  # (32,)
    b1z = np.zeros((128, 32), f)
    for zi in range(4):
        for ch in range(32):
            b1z[32 * zi + ch, :] = (ub[ch] + c1[ch] * zt[32 * zi:32 * zi + 32])

    ins = {
        "w1T": np.ascontiguousarray(sw1.T).astype(bf),
        "b1c": sb1[:, None].astype(f),
        "w2T": np.ascontiguousarray(sw2.T).astype(bf),
        "b2c": sb2[:, None].astype(f),
        "wU": np.ascontiguousarray(wU).astype(bf),
        "b1z": b1z,
        "l2T": np.kron(np.eye(4, dtype=f), mw2.T).astype(bf),
        "b2r": np.tile(mb2, 4)[:, None].astype(f),
        "l3T": np.concatenate([np.kron(np.eye(4, dtype=f), mw3.T)] * 2,
                              axis=1).astype(bf),
        "b3r": np.tile(mb3, 8)[:, None].astype(f),
        "l4T": np.kron(np.eye(8, dtype=f), mw4.T).astype(bf),
        "mb4c": np.full((128, 1), mb4[0], f),
    }
    in_maps = []
    for kk in range(N_CORES):
        xs = x[:, :, kk * HL:(kk + 1) * HL, :]
        xcore = np.ascontiguousarray(
            xs.transpose(1, 0, 2, 3).reshape(C, PIX)).astype(bf)
        in_maps.append({**ins, "x_sb": xcore})
    return in_maps


def run(trace=False, **inputs):
    if "nc" not in _CACHE:
        _CACHE["nc"] = _build_program()
    nc = _CACHE["nc"]
    in_maps = _host_inputs(**inputs)
    res = run_bass_kernel_spmd(nc, in_maps, list(range(N_CORES)), trace=trace)
    out = np.empty((B, D, H, W), np.float32)
    for kk in range(N_CORES):
        o = res.results[kk]["out_sd"].reshape(D, B, HL, W).transpose(1, 0, 2, 3)
        out[:, :, kk * HL:(kk + 1) * HL, :] = o
    return out, res


def kernel(**inputs):
    out, _ = run(trace=False, **inputs)
    return out


# revision 10
# speedup vs baseline: 2.3228x; 1.1005x over previous
"""Trainium2 Bass kernel for CompressionSDF (4,128,128,128) -> (4,128,128,128).

Structure of the computation:
  stage: 1x1-conv stack over (B,C=128,H,W): 128->64->32->16 (lrelu, lrelu, none)
  then per-voxel MLP over a z-broadcast 17-channel field: 17->32->32->16->1
  (lrelu x3, sigmoid), where channel 16 is a z linspace coordinate.

Sharding: H axis split across 8 cores (16 rows each). Per core 8192 pixels,
1,048,576 voxels.

Key observation: the voxel-MLP's first layer is h1 = lrelu(W1f@g + c1*z + b1)
where the z-dependence is a pure additive bias.  With a chunk layout of
[4 z-groups x 32 ch, 512 pixels] (z = 32*zi + c for chunk c), the z term is a
per-partition bias, so h1 never needs a matmul:
  U = (W1f@W3) @ f2  (+ fold of stage-3 bias) is computed once per pixel
  (replicated x4 over partition z-groups by tiling the stationary), and per
  chunk h1 = lrelu(U + b1z[:, c]) is produced by the DVE in two bf16 passes
  (tensor_scalar add; scalar_tensor_tensor max(0.01*q, q)).
L2/L3 use block-diagonal weights (4x32x32 / 4x32x16), L4 packs 8 rows per
chunk-pair and parks 4 pairs at 32-row offsets of one PSUM tile so a single
Sigmoid evacuates 8 chunks.  Output rows are 2KB-contiguous per z.
"""

import sys

sys.path.insert(0, "/opt/trn_rl_repo")

import numpy as np
from contextlib import ExitStack

import concourse.bass as bass
import concourse.tile as tile
from concourse import bacc, mybir
from concourse.bass_utils import run_bass_kernel_spmd

F32 = mybir.dt.float32
BF16 = mybir.dt.bfloat16
AF = mybir.ActivationFunctionType
ALU = mybir.AluOpType

N_CORES = 8
B, C, H, W, D = 4, 128, 128, 128, 128
HL = H // N_CORES            # h rows per core = 16
PIX = B * HL * W             # pixels per core = 8192
PB = 512                     # pixels per block
NPB = PIX // PB              # 16 pixel blocks
ALPHA = 0.01                 # LeakyReLU slope

_CACHE = {}


def _build_program(trace=False):
    nc = bacc.Bacc(
        "TRN2",
        target_bir_lowering=False,
        debug=False,
        enable_asserts=False,
        num_devices=N_CORES,
    )

    def din(name, shape, dt=F32):
        return nc.dram_tensor(name, list(shape), dt, kind="ExternalInput")

    x_d = din("x_sb", (C, PIX), BF16)
    w1_d = din("w1T", (128, 64), BF16)
    b1_d = din("b1c", (64, 1))
    w2_d = din("w2T", (64, 32), BF16)
    b2_d = din("b2c", (32, 1))
    wU_d = din("wU", (32, 128), BF16)
    b1z_d = din("b1z", (128, 32))
    l2_d = din("l2T", (128, 128), BF16)
    b2r_d = din("b2r", (128, 1))
    l3_d = din("l3T", (128, 128), BF16)
    b3r_d = din("b3r", (128, 1))
    b3f_d = din("b3f", (128, 1))
    l4_d = din("l4T", (128, 8), BF16)
    mb4_d = din("mb4c", (128, 1))
    out_d = nc.dram_tensor("out_sd", [D, PIX], F32, kind="ExternalOutput")

    with tile.TileContext(nc) as tc, ExitStack() as octx:
        cpool = octx.enter_context(tc.tile_pool(name="consts", bufs=1))

        def load(name, dram, shape, dt=F32):
            t = cpool.tile(list(shape), dt, name=name)
            nc.sync.dma_start(out=t[:], in_=dram[:])
            return t

        w1s = load("w1s", w1_d, (128, 64), BF16)
        b1s = load("b1s", b1_d, (64, 1))
        w2s = load("w2s", w2_d, (64, 32), BF16)
        b2s = load("b2s", b2_d, (32, 1))
        wUs = load("wUs", wU_d, (32, 128), BF16)
        b1zs = load("b1zs", b1z_d, (128, 32))
        l2s = load("l2s", l2_d, (128, 128), BF16)
        b2rs = load("b2rs", b2r_d, (128, 1))
        l3s = load("l3s", l3_d, (128, 128), BF16)
        b3rs = load("b3rs", b3r_d, (128, 1))
        b3fs = load("b3fs", b3f_d, (128, 1))
        l4s = load("l4s", l4_d, (128, 8), BF16)
        mb4s = load("mb4s", mb4_d, (128, 1))

        xt = cpool.tile([C, PIX], BF16, name="xt")

        # out rows: z = 32*zi + rr, rr = 8*g + 2*k + q;
        # psig partition = 32*k + 4*q + zi.  src keeps a plain contiguous
        # 4-partition range (zi), dst dim0 = zi with uniform 32-row stride.
        outz = out_d[:].rearrange("(z r) n -> r z n", z=4, r=32)

        with tc.tile_pool(name="psS", bufs=1, space="PSUM") as psS, \
             tc.tile_pool(name="ps1", bufs=2, space="PSUM") as ps1, \
             tc.tile_pool(name="ps2", bufs=2, space="PSUM") as ps2, \
             tc.tile_pool(name="ps3", bufs=1, space="PSUM") as ps3, \
             tc.tile_pool(name="f1p", bufs=2) as f1p, \
             tc.tile_pool(name="f2p", bufs=2) as f2p, \
             tc.tile_pool(name="u4p", bufs=2) as u4p, \
             tc.tile_pool(name="qp", bufs=3) as qp, \
             tc.tile_pool(name="h1p", bufs=3) as h1p, \
             tc.tile_pool(name="h2p", bufs=3) as h2p, \
             tc.tile_pool(name="h3p", bufs=3) as h3p, \
             tc.tile_pool(name="sgp", bufs=2) as sgp:
            for t in range(NPB):
                pbs = bass.ts(t, PB)
                # ---- stage convs + U for this pixel block ----
                nc.sync.dma_start(out=xt[:, pbs], in_=x_d[:, pbs])
                pa = psS.tile([64, PB], F32, name="pa", tag="st")
                nc.tensor.matmul(pa[:], w1s[:], xt[:, pbs], start=True, stop=True)
                f1 = f1p.tile([64, PB], BF16, name="f1")
                nc.scalar.activation(f1[:], pa[:], AF.Prelu, bias=b1s[:], alpha=ALPHA)
                pb2 = psS.tile([32, PB], F32, name="pb2", tag="st")
                nc.tensor.matmul(pb2[:], w2s[:], f1[:], start=True, stop=True)
                f2 = f2p.tile([32, PB], BF16, name="f2")
                nc.scalar.activation(f2[:], pb2[:], AF.Prelu, bias=b2s[:], alpha=ALPHA)
                pU = psS.tile([128, PB], F32, name="pU", tag="st")
                nc.tensor.matmul(pU[:], wUs[:], f2[:], start=True, stop=True)
                u4 = u4p.tile([128, PB], BF16, name="u4")
                nc.vector.tensor_copy(u4[:], pU[:])

                # ---- per-voxel MLP: 32 chunks of (512 pix x 4 z) ----
                psig = None
                p3 = None
                for u in range(16):          # chunk pairs
                    g, k = u // 4, u % 4
                    c0 = 2 * u
                    # h1 = lrelu(U + b1z[:, c]) on DVE, batched per pair:
                    # two bias-adds into one [128,1024] tile, one 1024-col
                    # max(0.01*q, q) pass
                    q2 = qp.tile([128, 2 * PB], BF16, name="q2")
                    nc.vector.tensor_scalar(q2[:, 0:PB], u4[:],
                                            b1zs[:, c0:c0 + 1],
                                            None, op0=ALU.add)
                    nc.vector.tensor_scalar(q2[:, PB:2 * PB], u4[:],
                                            b1zs[:, c0 + 1:c0 + 2],
                                            None, op0=ALU.add)
                    h12 = h1p.tile([128, 2 * PB], BF16, name="h12")
                    nc.vector.scalar_tensor_tensor(h12[:], q2[:], ALPHA, q2[:],
                                                   op0=ALU.mult, op1=ALU.max)
                    # L2 (block-diag): two matmuls into one 2-bank PSUM tile,
                    # one 1024-col ACT Prelu evac
                    p2 = ps1.tile([128, 2 * PB], F32, name="p2")
                    nc.tensor.matmul(p2[:, 0:PB], l2s[:], h12[:, 0:PB],
                                     start=True, stop=True)
                    nc.tensor.matmul(p2[:, PB:2 * PB], l2s[:], h12[:, PB:2 * PB],
                                     start=True, stop=True)
                    h2 = h2p.tile([128, 2 * PB], BF16, name="h2")
                    nc.scalar.activation(h2[:], p2[:], AF.Prelu, bias=b2rs[:],
                                         alpha=ALPHA)
                    # L3: pair members share one PSUM tile via column groups
                    p3 = ps2.tile([128, PB], F32, name="p3")
                    nc.tensor.matmul(p3[0:64, :], l3s[:, 0:64], h2[:, 0:PB],
                                     start=True, stop=True,
                                     tile_position=(0, 0))
                    nc.tensor.matmul(p3[64:128, :], l3s[:, 64:128],
                                     h2[:, PB:2 * PB],
                                     start=True, stop=True,
                                     tile_position=(0, 64))
                    h3 = h3p.tile([128, PB], BF16, name="h3")
                    nc.scalar.activation(h3[:], p3[:], AF.Prelu, bias=b3rs[:],
                                         alpha=ALPHA)
                    # L4: park pair u at rows 32k of the group's PSUM tile
                    if k == 0:
                        psig = ps3.tile([128, PB], F32, name="psig")
                    nc.tensor.matmul(psig[32 * k:32 * k + 8, :], l4s[:], h3[:],
                                     start=True, stop=True,
                                     tile_position=(0, 32 * k))
                    if k == 3:
                        sg = sgp.tile([128, PB], F32, name="sg")
                        nc.scalar.activation(sg[:], psig[:], AF.Sigmoid,
                                             bias=mb4s[:])
                        for kk in range(4):
                            for q2 in range(2):
                                rr = 8 * g + 2 * kk + q2
                                nc.sync.dma_start(
                                    out=outz[rr, :, pbs],
                                    in_=sg[32 * kk + 4 * q2:
                                           32 * kk + 4 * q2 + 4, :])

    nc.compile()
    return nc


def _host_inputs(x, sw1, sb1, sw2, sb2, sw3, sb3,
                 mw1, mb1, mw2, mb2, mw3, mb3, mw4, mb4):
    import ml_dtypes
    f = np.float32
    bf = ml_dtypes.bfloat16
    zt = np.linspace(-1.0, 1.0, D, dtype=np.float64)
    c1 = mw1[:, 16].astype(np.float64)          # z-coordinate column of mw1
    W1f = mw1[:, :16].astype(np.float64)

    # U = (W1f @ sw3) @ f2 ; stationary [32, 128] tiled x4 over z-groups
    WU = (W1f @ sw3.astype(np.float64))          # (32, 16+)? sw3 is (16,32)
    wU = np.tile(WU.T, (1, 4)).astype(f)         # (32, 128)

    # b1z[(zi,ch), c] = (W1f@sb3 + mb1)[ch] + c1[ch] * z(32*zi + c)
    ub = (W1f @ sb3.astype(np.float64) + mb1.astype(np.float64))  # (32,)
    b1z = np.zeros((128, 32), f)
    for zi in range(4):
        for ch in range(32):
            b1z[32 * zi + ch, :] = (ub[ch] + c1[ch] * zt[32 * zi:32 * zi + 32])

    ins = {
        "w1T": np.ascontiguousarray(sw1.T).astype(bf),
        "b1c": sb1[:, None].astype(f),
        "w2T": np.ascontiguousarray(sw2.T).astype(bf),
        "b2c": sb2[:, None].astype(f),
        "wU": np.ascontiguousarray(wU).astype(bf),
        "b1z": b1z,
        "l2T": np.kron(np.eye(4, dtype=f), mw2.T).astype(bf),
        "b2r": np.tile(mb2, 4)[:, None].astype(f),
        "l3T": np.concatenate([np.kron(np.eye(4, dtype=f), mw3.T)] * 2,
                              axis=1).astype(bf),
        "b3r": np.tile(mb3, 8)[:, None].astype(f),
        "b3f": np.concatenate([np.tile(mb3, 4),
                               np.tile(mb3 + mw3 @ mb2, 4)])[:, None].astype(f),
        "l4T": np.kron(np.eye(8, dtype=f), mw4.T).astype(bf),
        "mb4c": np.full((128, 1), mb4[0], f),
    }
    in_maps = []
    for kk in range(N_CORES):
        xs = x[:, :, kk * HL:(kk + 1) * HL, :]
        xcore = np.ascontiguousarray(
            xs.transpose(1, 0, 2, 3).reshape(C, PIX)).astype(bf)
        in_maps.append({**ins, "x_sb": xcore})
    return in_maps


def run(trace=False, **inputs):
    if "nc" not in _CACHE:
        _CACHE["nc"] = _build_program()
    nc = _CACHE["nc"]
    in_maps = _host_inputs(**inputs)
    res = run_bass_kernel_spmd(nc, in_maps, list(range(N_CORES)), trace=trace)
    out = np.empty((B, D, H, W), np.float32)
    for kk in range(N_CORES):
        o = res.results[kk]["out_sd"].reshape(D, B, HL, W).transpose(1, 0, 2, 3)
        out[:, :, kk * HL:(kk + 1) * HL, :] = o
    return out, res


def kernel(**inputs):
    out, _ = run(trace=False, **inputs)
    return out


# revision 12
# speedup vs baseline: 2.3571x; 1.0148x over previous
"""Trainium2 Bass kernel for CompressionSDF (4,128,128,128) -> (4,128,128,128).

Structure of the computation:
  stage: 1x1-conv stack over (B,C=128,H,W): 128->64->32->16 (lrelu, lrelu, none)
  then per-voxel MLP over a z-broadcast 17-channel field: 17->32->32->16->1
  (lrelu x3, sigmoid), where channel 16 is a z linspace coordinate.

Sharding: H axis split across 8 cores (16 rows each). Per core 8192 pixels,
1,048,576 voxels.

Key observation: the voxel-MLP's first layer is h1 = lrelu(W1f@g + c1*z + b1)
where the z-dependence is a pure additive bias.  With a chunk layout of
[4 z-groups x 32 ch, 512 pixels] (z = 32*zi + c for chunk c), the z term is a
per-partition bias, so h1 never needs a matmul:
  U = (W1f@W3) @ f2  (+ fold of stage-3 bias) is computed once per pixel
  (replicated x4 over partition z-groups by tiling the stationary), and per
  chunk h1 = lrelu(U + b1z[:, c]) is produced by the DVE in two bf16 passes
  (tensor_scalar add; scalar_tensor_tensor max(0.01*q, q)).
L2/L3 use block-diagonal weights (4x32x32 / 4x32x16), L4 packs 8 rows per
chunk-pair and parks 4 pairs at 32-row offsets of one PSUM tile so a single
Sigmoid evacuates 8 chunks.  Output rows are 2KB-contiguous per z.
"""

import sys

sys.path.insert(0, "/opt/trn_rl_repo")

import numpy as np
from contextlib import ExitStack

import concourse.bass as bass
import concourse.tile as tile
from concourse import bacc, mybir
from concourse.bass_utils import run_bass_kernel_spmd

F32 = mybir.dt.float32
BF16 = mybir.dt.bfloat16
AF = mybir.ActivationFunctionType
ALU = mybir.AluOpType

N_CORES = 8
B, C, H, W, D = 4, 128, 128, 128, 128
HL = H // N_CORES            # h rows per core = 16
PIX = B * HL * W             # pixels per core = 8192
PB = 512                     # pixels per block
NPB = PIX // PB              # 16 pixel blocks
ALPHA = 0.01                 # LeakyReLU slope

_CACHE = {}


def _build_program(trace=False):
    nc = bacc.Bacc(
        "TRN2",
        target_bir_lowering=False,
        debug=False,
        enable_asserts=False,
        num_devices=N_CORES,
    )

    def din(name, shape, dt=F32):
        return nc.dram_tensor(name, list(shape), dt, kind="ExternalInput")

    x_d = din("x_sb", (C, PIX), BF16)
    w1_d = din("w1T", (128, 64), BF16)
    b1_d = din("b1c", (64, 1))
    w2_d = din("w2T", (64, 32), BF16)
    b2_d = din("b2c", (32, 1))
    wU_d = din("wU", (32, 128), BF16)
    b1z_d = din("b1z", (128, 32))
    l2_d = din("l2T", (128, 128), BF16)
    b2r_d = din("b2r", (128, 1))
    l3_d = din("l3T", (128, 128), BF16)
    b3r_d = din("b3r", (128, 1))
    b3f_d = din("b3f", (128, 1))
    l4_d = din("l4T", (128, 8), BF16)
    mb4_d = din("mb4c", (128, 1))
    out_d = nc.dram_tensor("out_sd", [D, PIX], F32, kind="ExternalOutput")

    with tile.TileContext(nc) as tc, ExitStack() as octx:
        cpool = octx.enter_context(tc.tile_pool(name="consts", bufs=1))

        def load(name, dram, shape, dt=F32):
            t = cpool.tile(list(shape), dt, name=name)
            nc.sync.dma_start(out=t[:], in_=dram[:])
            return t

        w1s = load("w1s", w1_d, (128, 64), BF16)
        b1s = load("b1s", b1_d, (64, 1))
        w2s = load("w2s", w2_d, (64, 32), BF16)
        b2s = load("b2s", b2_d, (32, 1))
        wUs = load("wUs", wU_d, (32, 128), BF16)
        b1zs = load("b1zs", b1z_d, (128, 32))
        l2s = load("l2s", l2_d, (128, 128), BF16)
        b2rs = load("b2rs", b2r_d, (128, 1))
        l3s = load("l3s", l3_d, (128, 128), BF16)
        b3rs = load("b3rs", b3r_d, (128, 1))
        b3fs = load("b3fs", b3f_d, (128, 1))
        l4s = load("l4s", l4_d, (128, 8), BF16)
        mb4s = load("mb4s", mb4_d, (128, 1))

        xt = cpool.tile([C, PIX], BF16, name="xt")

        # out rows: z = 32*zi + rr, rr = 8*g + 2*k + q;
        # psig partition = 32*k + 4*q + zi.  src keeps a plain contiguous
        # 4-partition range (zi), dst dim0 = zi with uniform 32-row stride.
        outz = out_d[:].rearrange("(z r) n -> r z n", z=4, r=32)

        with tc.tile_pool(name="psS", bufs=1, space="PSUM") as psS, \
             tc.tile_pool(name="ps1", bufs=2, space="PSUM") as ps1, \
             tc.tile_pool(name="ps2", bufs=1, space="PSUM") as ps2, \
             tc.tile_pool(name="ps3", bufs=1, space="PSUM") as ps3, \
             tc.tile_pool(name="f1p", bufs=2) as f1p, \
             tc.tile_pool(name="f2p", bufs=2) as f2p, \
             tc.tile_pool(name="u4p", bufs=2) as u4p, \
             tc.tile_pool(name="qp", bufs=3) as qp, \
             tc.tile_pool(name="h1p", bufs=3) as h1p, \
             tc.tile_pool(name="h2p", bufs=3) as h2p, \
             tc.tile_pool(name="h3p", bufs=3) as h3p, \
             tc.tile_pool(name="sgp", bufs=2) as sgp:
            for t in range(NPB):
                pbs = bass.ts(t, PB)
                # ---- stage convs + U for this pixel block ----
                nc.sync.dma_start(out=xt[:, pbs], in_=x_d[:, pbs])
                pa = psS.tile([64, PB], F32, name="pa", tag="st")
                nc.tensor.matmul(pa[:], w1s[:], xt[:, pbs], start=True, stop=True)
                f1 = f1p.tile([64, PB], BF16, name="f1")
                nc.scalar.activation(f1[:], pa[:], AF.Prelu, bias=b1s[:], alpha=ALPHA)
                pb2 = psS.tile([32, PB], F32, name="pb2", tag="st")
                nc.tensor.matmul(pb2[:], w2s[:], f1[:], start=True, stop=True)
                f2 = f2p.tile([32, PB], BF16, name="f2")
                nc.scalar.activation(f2[:], pb2[:], AF.Prelu, bias=b2s[:], alpha=ALPHA)
                pU = psS.tile([128, PB], F32, name="pU", tag="st")
                nc.tensor.matmul(pU[:], wUs[:], f2[:], start=True, stop=True)
                u4 = u4p.tile([128, PB], BF16, name="u4")
                nc.vector.tensor_copy(u4[:], pU[:])

                # ---- per-voxel MLP: 32 chunks of (512 pix x 4 z) ----
                psig = None
                p3 = None
                for u in range(16):          # chunk pairs
                    g, k = u // 4, u % 4
                    w = u % 2                    # pair-in-group for h3 batching
                    c0 = 2 * u
                    # h1 = lrelu(U + b1z[:, c]) on DVE, batched per pair:
                    # two bias-adds into one [128,1024] tile, one 1024-col
                    # max(0.01*q, q) pass
                    q2 = qp.tile([128, 2 * PB], BF16, name="q2")
                    nc.vector.tensor_scalar(q2[:, 0:PB], u4[:],
                                            b1zs[:, c0:c0 + 1],
                                            None, op0=ALU.add)
                    nc.vector.tensor_scalar(q2[:, PB:2 * PB], u4[:],
                                            b1zs[:, c0 + 1:c0 + 2],
                                            None, op0=ALU.add)
                    h12 = h1p.tile([128, 2 * PB], BF16, name="h12")
                    nc.vector.scalar_tensor_tensor(h12[:], q2[:], ALPHA, q2[:],
                                                   op0=ALU.mult, op1=ALU.max)
                    # L2 (block-diag): two matmuls into one 2-bank PSUM tile,
                    # one 1024-col ACT Prelu evac
                    p2 = ps1.tile([128, 2 * PB], F32, name="p2")
                    nc.tensor.matmul(p2[:, 0:PB], l2s[:], h12[:, 0:PB],
                                     start=True, stop=True)
                    nc.tensor.matmul(p2[:, PB:2 * PB], l2s[:], h12[:, PB:2 * PB],
                                     start=True, stop=True)
                    h2 = h2p.tile([128, 2 * PB], BF16, name="h2")
                    nc.scalar.activation(h2[:], p2[:], AF.Prelu, bias=b2rs[:],
                                         alpha=ALPHA)
                    # L3: two pairs share one 2-bank PSUM tile; one
                    # 1024-col ACT Prelu evac per 2 pairs
                    if w == 0:
                        p3 = ps2.tile([128, 2 * PB], F32, name="p3")
                    nc.tensor.matmul(p3[0:64, w * PB:(w + 1) * PB],
                                     l3s[:, 0:64], h2[:, 0:PB],
                                     start=True, stop=True,
                                     tile_position=(0, 0))
                    nc.tensor.matmul(p3[64:128, w * PB:(w + 1) * PB],
                                     l3s[:, 64:128], h2[:, PB:2 * PB],
                                     start=True, stop=True,
                                     tile_position=(0, 64))
                    if w == 0:
                        continue
                    h3 = h3p.tile([128, 2 * PB], BF16, name="h3")
                    nc.scalar.activation(h3[:], p3[:], AF.Prelu, bias=b3rs[:],
                                         alpha=ALPHA)
                    # L4: park pairs at rows 32k of the group's PSUM tile
                    for w2 in range(2):
                        u2 = u - 1 + w2
                        k2 = u2 % 4
                        if k2 == 0:
                            psig = ps3.tile([128, PB], F32, name="psig")
                        nc.tensor.matmul(psig[32 * k2:32 * k2 + 8, :], l4s[:],
                                         h3[:, w2 * PB:(w2 + 1) * PB],
                                         start=True, stop=True,
                                         tile_position=(0, 32 * k2))
                    if k == 3:
                        sg = sgp.tile([128, PB], F32, name="sg")
                        nc.scalar.activation(sg[:], psig[:], AF.Sigmoid,
                                             bias=mb4s[:])
                        for kk in range(4):
                            for q2 in range(2):
                                rr = 8 * g + 2 * kk + q2
                                nc.sync.dma_start(
                                    out=outz[rr, :, pbs],
                                    in_=sg[32 * kk + 4 * q2:
                                           32 * kk + 4 * q2 + 4, :])

    nc.compile()
    return nc


def _host_inputs(x, sw1, sb1, sw2, sb2, sw3, sb3,
                 mw1, mb1, mw2, mb2, mw3, mb3, mw4, mb4):
    import ml_dtypes
    f = np.float32
    bf = ml_dtypes.bfloat16
    zt = np.linspace(-1.0, 1.0, D, dtype=np.float64)
    c1 = mw1[:, 16].astype(np.float64)          # z-coordinate column of mw1
    W1f = mw1[:, :16].astype(np.float64)

    # U = (W1f @ sw3) @ f2 ; stationary [32, 128] tiled x4 over z-groups
    WU = (W1f @ sw3.astype(np.float64))          # (32, 16+)? sw3 is (16,32)
    wU = np.tile(WU.T, (1, 4)).astype(f)         # (32, 128)

    # b1z[(zi,ch), c] = (W1f@sb3 + mb1)[ch] + c1[ch] * z(32*zi + c)
    ub = (W1f @ sb3.astype(np.float64) + mb1.astype(np.float64))  # (32,)
    b1z = np.zeros((128, 32), f)
    for zi in range(4):
        for ch in range(32):
            b1z[32 * zi + ch, :] = (ub[ch] + c1[ch] * zt[32 * zi:32 * zi + 32])

    ins = {
        "w1T": np.ascontiguousarray(sw1.T).astype(bf),
        "b1c": sb1[:, None].astype(f),
        "w2T": np.ascontiguousarray(sw2.T).astype(bf),
        "b2c": sb2[:, None].astype(f),
        "wU": np.ascontiguousarray(wU).astype(bf),
        "b1z": b1z,
        "l2T": np.kron(np.eye(4, dtype=f), mw2.T).astype(bf),
        "b2r": np.tile(mb2, 4)[:, None].astype(f),
        "l3T": np.concatenate([np.kron(np.eye(4, dtype=f), mw3.T)] * 2,
                              axis=1).astype(bf),
        "b3r": np.tile(mb3, 8)[:, None].astype(f),
        "b3f": np.concatenate([np.tile(mb3, 4),
                               np.tile(mb3 + mw3 @ mb2, 4)])[:, None].astype(f),
        "l4T": np.kron(np.eye(8, dtype=f), mw4.T).astype(bf),
        "mb4c": np.full((128, 1), mb4[0], f),
    }
    in_maps = []
    for kk in range(N_CORES):
        xs = x[:, :, kk * HL:(kk + 1) * HL, :]
        xcore = np.ascontiguousarray(
            xs.transpose(1, 0, 2, 3).reshape(C, PIX)).astype(bf)
        in_maps.append({**ins, "x_sb": xcore})
    return in_maps


def run(trace=False, **inputs):
    if "nc" not in _CACHE:
        _CACHE["nc"] = _build_program()
    nc = _CACHE["nc"]
    in_maps = _host_inputs(**inputs)
    res = run_bass_kernel_spmd(nc, in_maps, list(range(N_CORES)), trace=trace)
    out = np.empty((B, D, H, W), np.float32)
    for kk in range(N_CORES):
        o = res.results[kk]["out_sd"].reshape(D, B, HL, W).transpose(1, 0, 2, 3)
        out[:, :, kk * HL:(kk + 1) * HL, :] = o
    return out, res


def kernel(**inputs):
    out, _ = run(trace=False, **inputs)
    return out
